# revision 22
# baseline (speedup 1.0000x reference)
"""DirectedGraphConvolution Trainium2 kernel (bf16, software-pipelined).

Per batch element b (one per NeuronCore, 8 total, data-parallel):
    N_e = H @ W                          [n, dout]
    T1  = G  @ N_e ; T2 = G.T @ N_e
    rs  = G.sum(-1); cs = G.sum(-2)
    out = [ relu(0.5*(T1 + T2)),
            relu(G.T @ (T1 / rs[:,None])),
            relu(G  @ (T2 / cs[:,None])) ]

All matmuls run in bf16 (f32 PSUM, ~1 cyc/row; scale-relative error
~3e-3, well inside the 2e-2 gate).  G streams in f32 on the sync queue
through a 4-slot staging ring (H borrows the first two slots).  Per
tile, the f32->bf16 cast is split ACT/GpSimd and runs two tiles ahead;
the PE transposes all 16 blocks (bf16 transposes are 1 cyc/row) into a
persistent G.T strip tensor via two 8-block strided PSUM->SBUF copies
on DVE (16-bit DVE copies run 2 elem/cyc), and B1 = G @ [ones|N_e]
follows three tiles behind, keeping the PE busy through the whole DMA
window.  B1's ones column yields rs; rs also becomes a diag(rs) bf16
block (DVE scales the identity) and T1' = T1/rs lands next to N_e in
the shared moving tensor nm = [ones | N_e | T1'].

Post-load, passes A and C share their stationaries (natural G blocks),
so they run fused with the 512-wide moving [N_e | T1'] -- wide enough
to hide the 115 ns LDWEIGHTS per stationary that rate-limits 256-wide
matmuls.  Epilogue per column tile: cs from DVE strip reductions
(pipelined two tiles ahead), T2' = T2/cs, then one diag(rs) @ T1'
matmul accumulates T1 into the PSUM tile so out1 = relu(0.5*psum)
needs no DVE add.  B2 = G @ T2' reuses the strips (G is transposed
exactly once).  Outputs stream per-tile on the sync/gpsimd queues.
"""

import numpy as np
import concourse.bass as bass
import concourse.mybir as mybir
import concourse.tile as tile
from concourse import bacc
from concourse.bass_utils import run_bass_kernel_spmd
from concourse.masks import make_identity

F32 = mybir.dt.float32
BF16 = mybir.dt.bfloat16
COPY = mybir.ActivationFunctionType.Copy
RELU = mybir.ActivationFunctionType.Relu
AX = mybir.AxisListType.X

P = 128
B = 8
N = 2048
NO = N // P            # 16 row tiles
DIN = 256
DOUT = 256
KO = DIN // P          # 2 k tiles for H @ W
W3 = 3 * DOUT
D2 = DOUT + 2          # moving block width with ones column


def build():
    nc = bacc.Bacc("TRN2", target_bir_lowering=False)
    G = nc.declare_dram_parameter("G", [N, N], F32, isOutput=False)
    H = nc.declare_dram_parameter("H", [N, DIN], F32, isOutput=False)
    W = nc.declare_dram_parameter("W", [DIN, DOUT], F32, isOutput=False)
    out = nc.declare_dram_parameter("out", [N, W3], F32, isOutput=True)

    G_r = G.rearrange("(o p) j -> p o j", p=P)
    H_r = H.rearrange("(o p) d -> p o d", p=P)
    W_r = W.rearrange("(o p) d -> p o d", p=P)
    out_r = out.rearrange("(o p) d -> p o d", p=P)

    with tile.TileContext(nc) as tc:
        with (
            tc.tile_pool(name="const", bufs=1) as const,
            tc.tile_pool(name="gn", bufs=1) as gn,
            tc.tile_pool(name="gt", bufs=1) as gt,
            tc.tile_pool(name="gstage", bufs=4) as gstage,
            tc.tile_pool(name="nmp", bufs=1) as nmp,
            tc.tile_pool(name="t2pp", bufs=1) as t2pp,
            tc.tile_pool(name="stage", bufs=3) as stage,
        ):
            # ---- constants ----
            ident_f32 = const.tile([P, P], F32)
            make_identity(nc, ident_f32)
            ident16 = const.tile([P, P], BF16)
            nc.vector.tensor_copy(ident16, ident_f32)
            rs_sb = const.tile([P, NO, 1], F32)
            rsinv = const.tile([P, NO, 1], F32)
            csinv = const.tile([P, NO, 1], F32)

            # ---- persistent bf16 tensors ----
            g16 = [gn.tile([P, N], BF16, tag=f"g{o}", name=f"g{o}") for o in range(NO)]
            strips = gt.tile([P, NO, N], BF16)     # [col-in-block, jt, row]
            # moving operand per k tile: [ones(0:2) | N_e(2:258) | T1'(258:514)]
            nm = nmp.tile([P, NO, 514], BF16)
            nc.vector.memset(nm[:, :, 0:2], 1.0)
            t2p = t2pp.tile([P, NO, DOUT], BF16)
            diag16 = t2pp.tile([P, NO, P], BF16)   # diag(rs) blocks
            cs_sb = const.tile([P, NO, 1], F32)

            # ---- input DMAs: W, H first (sync queue head), then G.
            # H stages inside the G ring (2 slots, recycled for G after). ----
            hs = []
            for i in range(2):
                h_sl = gstage.tile([P, N], F32, tag="gs", name=f"hs{i}")
                nc.sync.dma_start(
                    h_sl.rearrange("p (a b) -> p a b", a=8),
                    H_r[:, 8 * i:8 * (i + 1), :],
                )
                hs.append(h_sl)
            w_st = const.tile([P, KO, DOUT], F32)
            nc.sync.dma_start(w_st, W_r)

            gs_tiles = []
            for o in range(NO):
                gs = gstage.tile([P, N], F32, tag="gs", name=f"gs{o}")
                nc.sync.dma_start(gs[:, 0:N // 2], G_r[:, o, 0:N // 2])
                nc.sync.dma_start(gs[:, N // 2:N], G_r[:, o, N // 2:N])
                gs_tiles.append(gs)

            # ---- H @ W -> nm[:, :, 0:DOUT] ----
            w16 = const.tile([P, KO, DOUT], BF16)
            nc.vector.tensor_copy(w16, w_st)
            lpools = tc.tile_pool(name="psT", bufs=1, space="PSUM")
            psTp = lpools.__enter__()
            lpools2 = tc.tile_pool(name="psB1", bufs=2, space="PSUM")
            psB1 = lpools2.__enter__()
            with (
                tc.tile_pool(name="h16p", bufs=4) as h16p,
                tc.tile_pool(name="htp", bufs=3) as htp,
                tc.tile_pool(name="ps_h", bufs=1, space="PSUM") as ps_h,
                tc.tile_pool(name="ps_ne", bufs=2, space="PSUM") as ps_ne,
            ):
                psh = ps_h.tile([P, 8, P], BF16)
                hts = {}
                h16s = {}
                for t in range(NO + 1):
                    if t < NO:
                        if t % 2 == 0:   # cast two H tiles at once, alt engines
                            h16_t = h16p.tile([P, 2, DIN], BF16, tag="h16")
                            sl = hs[t // 8].rearrange("p (a b) -> p a b", a=8)
                            src_ap = sl[:, t % 8:t % 8 + 2, :]
                            if t % 4 == 0:
                                nc.scalar.copy(h16_t, src_ap)
                            else:
                                nc.vector.tensor_copy(h16_t, src_ap)
                            h16s[t] = h16_t
                        h16_c = h16s[t - t % 2][:, t % 2, :]
                        ht_t = htp.tile([P, KO, P], BF16, tag="ht")
                        s = (2 * t) % 8
                        for kt in range(KO):
                            nc.tensor.transpose(
                                psh[:, s + kt, :],
                                h16_c[:, kt * P:(kt + 1) * P],
                                ident16,
                            )
                        if t % 2 == 0:
                            nc.vector.tensor_copy(ht_t, psh[:, s:s + 2, :])
                        else:
                            nc.scalar.copy(ht_t, psh[:, s:s + 2, :])
                        hts[t] = ht_t
                    if t >= 1:
                        u = t - 1
                        ht_u = hts.pop(u)
                        pne = ps_ne.tile([P, DOUT], F32, tag="pne")
                        for kt in range(KO):
                            nc.tensor.matmul(
                                pne,
                                ht_u[:, kt, :],
                                w16[:, kt, :],
                                start=(kt == 0),
                                stop=(kt == KO - 1),
                            )
                        if u % 2 == 0:
                            nc.vector.tensor_copy(nm[:, u, 2:D2], pne)
                        else:
                            nc.scalar.copy(nm[:, u, 2:D2], pne)

            # ---- phase L: per arriving G tile: cast, transposes, B1 ----
            if True:
                psT = psTp.tile([P, 16, P], BF16)  # 16 transpose slots = 2 banks

                def b1_pass(u):
                    # [rs | T1] = G[u rows, :] @ [ones | N_e]
                    pb1 = psB1.tile([P, D2], F32, tag="pb1")
                    for jm in range(NO):
                        nc.tensor.matmul(
                            pb1,
                            strips[:, jm, u * P:(u + 1) * P],
                            nm[:, jm, 0:D2],
                            start=(jm == 0),
                            stop=(jm == NO - 1),
                        )
                    nc.vector.reciprocal(rsinv[:, u, :], pb1[:, 0:1])
                    # diag(rs) block for the fused pass's T1*rs accumulation
                    nc.vector.tensor_scalar_mul(
                        diag16[:, u, :], ident16, pb1[:, 0:1]
                    )
                    # nm[.., 258:514] = T1' = T1 * rsinv  (ACT, scale port)
                    nc.scalar.activation(
                        nm[:, u, D2:D2 + DOUT], pb1[:, 2:D2], COPY,
                        scale=rsinv[:, u, 0:1],
                    )

                CA = 12 * P   # ACT casts [0:1536], GpSimd the rest

                def cast_tile(o):
                    nc.scalar.copy(g16[o][:, 0:CA], gs_tiles[o][:, 0:CA])
                    nc.gpsimd.tensor_copy(g16[o][:, CA:N], gs_tiles[o][:, CA:N])

                cast_tile(0)
                cast_tile(1)
                for o in range(NO):
                    if o + 2 < NO:
                        cast_tile(o + 2)
                    for q in range(2):       # 8 transposes + 1 octo copy, x2
                        for jt in range(8 * q, 8 * q + 8):
                            nc.tensor.transpose(
                                psT[:, jt, :],
                                g16[o][:, jt * P:(jt + 1) * P],
                                ident16,
                            )
                        src = psT[:, 8 * q:8 * q + 8, :]
                        dst = strips[:, 8 * q:8 * q + 8, o * P:(o + 1) * P]
                        nc.vector.tensor_copy(dst, src)
                    if o >= 3:
                        b1_pass(o - 3)
                for u in range(NO - 3, NO):
                    b1_pass(u)

            # ---- fused pass: [T2+diag(rs)@T1' | out2raw] = G.T @ [N_e|T1'] ----
            # cs comes from DVE strip reductions, pipelined two tiles ahead.
            with (
                tc.tile_pool(name="psAC", bufs=4, space="PSUM") as psAC,
            ):
                def cs_reduce(j):
                    nc.vector.reduce_sum(cs_sb[:, j, :], strips[:, j, :], axis=AX)

                cs_reduce(0)
                cs_reduce(1)

                def ac_finish(j, pa):
                    # pa[:,0:DOUT] += T1*rs  (diag matmul; after t2p read T2)
                    nc.tensor.matmul(
                        pa[:, 0:DOUT],
                        diag16[:, j, :],
                        nm[:, j, D2:D2 + DOUT],
                        start=False,
                        stop=True,
                        skip_group_check=True,
                    )
                    o1 = stage.tile([P, DOUT], F32, tag="o1")
                    nc.scalar.activation(o1, pa[:, 0:DOUT], RELU, scale=0.5)
                    nc.gpsimd.dma_start(out_r[:, j, 0:DOUT], o1)
                    o2 = stage.tile([P, DOUT], F32, tag="o2")
                    nc.scalar.activation(o2, pa[:, DOUT:2 * DOUT], RELU)
                    nc.sync.dma_start(out_r[:, j, DOUT:2 * DOUT], o2)

                pas = {}
                for jt in range(NO):
                    pa = psAC.tile([P, 2 * DOUT], F32, tag="pa")
                    for kt in range(NO):
                        nc.tensor.matmul(
                            pa,
                            g16[kt][:, jt * P:(jt + 1) * P],
                            nm[:, kt, 2:2 + 2 * DOUT],
                            start=(kt == 0),
                            stop=(kt == NO - 1),
                        )
                    pas[jt] = pa
                    nc.vector.reciprocal(csinv[:, jt, :], cs_sb[:, jt, 0:1])
                    nc.vector.tensor_scalar_mul(
                        t2p[:, jt, :], pa[:, 0:DOUT], csinv[:, jt, 0:1]
                    )
                    if jt + 2 < NO:
                        cs_reduce(jt + 2)
                    if jt >= 1:
                        ac_finish(jt - 1, pas.pop(jt - 1))
                ac_finish(NO - 1, pas.pop(NO - 1))

            lpools2.__exit__(None, None, None)
            lpools.__exit__(None, None, None)

            # ---- pass B2: out3 = relu(G @ T2') ----
            with tc.tile_pool(name="psB2", bufs=5, space="PSUM") as psB2:
                for it in range(NO):
                    pb = psB2.tile([P, DOUT], F32, tag="pb")
                    for jt in range(NO):
                        nc.tensor.matmul(
                            pb,
                            strips[:, jt, it * P:(it + 1) * P],
                            t2p[:, jt, :],
                            start=(jt == 0),
                            stop=(jt == NO - 1),
                        )
                    o3 = stage.tile([P, DOUT], F32, tag="o3")
                    nc.scalar.activation(o3, pb, RELU)
                    nc.sync.dma_start(out_r[:, it, 2 * DOUT:W3], o3)

    nc.compile()
    return nc


_NC = None


def _get_nc():
    global _NC
    if _NC is None:
        _NC = build()
    return _NC


def run(inputs: dict, trace: bool = False):
    """Run on 8 cores; returns (stacked_out [B,N,W3], BassKernelResults)."""
    H, G, W = inputs["H"], inputs["G"], inputs["W"]
    H = np.ascontiguousarray(H, dtype=np.float32)
    G = np.ascontiguousarray(G, dtype=np.float32)
    W = np.ascontiguousarray(W, dtype=np.float32)
    in_maps = [
        {"G": np.ascontiguousarray(G[b]), "H": np.ascontiguousarray(H[b]), "W": W}
        for b in range(B)
    ]
    nc = _get_nc()
    res = run_bass_kernel_spmd(nc, in_maps, core_ids=list(range(B)), trace=trace)
    out = np.stack([res.results[b]["out"] for b in range(B)], axis=0)
    return out, res


def kernel(H, G, W):
    out, _ = run({"H": H, "G": G, "W": W})
    return out


# revision 23
# speedup vs baseline: 1.0034x; 1.0034x over previous
"""DirectedGraphConvolution Trainium2 kernel (bf16, software-pipelined).

Per batch element b (one per NeuronCore, 8 total, data-parallel):
    N_e = H @ W                          [n, dout]
    T1  = G  @ N_e ; T2 = G.T @ N_e
    rs  = G.sum(-1); cs = G.sum(-2)
    out = [ relu(0.5*(T1 + T2)),
            relu(G.T @ (T1 / rs[:,None])),
            relu(G  @ (T2 / cs[:,None])) ]

All matmuls run in bf16 (f32 PSUM, ~1 cyc/row; scale-relative error
~3e-3, well inside the 2e-2 gate).  G streams in f32 on the sync queue
through a 4-slot staging ring (H borrows the first two slots).  Per
tile, the f32->bf16 cast is split ACT/GpSimd and runs two tiles ahead;
the PE transposes all 16 blocks (bf16 transposes are 1 cyc/row) into a
persistent G.T strip tensor via two 8-block strided PSUM->SBUF copies
on DVE (16-bit DVE copies run 2 elem/cyc), and B1 = G @ [ones|N_e]
follows three tiles behind, keeping the PE busy through the whole DMA
window.  B1's ones column yields rs; rs also becomes a diag(rs) bf16
block (DVE scales the identity) and T1' = T1/rs lands next to N_e in
the shared moving tensor nm = [ones | N_e | T1'].

Post-load, passes A and C share their stationaries (natural G blocks),
so they run fused with the 512-wide moving [N_e | T1'] -- wide enough
to hide the 115 ns LDWEIGHTS per stationary that rate-limits 256-wide
matmuls.  Epilogue per column tile: cs from DVE strip reductions
(pipelined two tiles ahead), T2' = T2/cs, then one diag(rs) @ T1'
matmul accumulates T1 into the PSUM tile so out1 = relu(0.5*psum)
needs no DVE add.  B2 = G @ T2' reuses the strips (G is transposed
exactly once).  Outputs stream per-tile on the sync/gpsimd queues.
"""

import numpy as np
import concourse.bass as bass
import concourse.mybir as mybir
import concourse.tile as tile
from concourse import bacc
from concourse.bass_utils import run_bass_kernel_spmd
from concourse.masks import make_identity

F32 = mybir.dt.float32
BF16 = mybir.dt.bfloat16
COPY = mybir.ActivationFunctionType.Copy
RELU = mybir.ActivationFunctionType.Relu
AX = mybir.AxisListType.X

P = 128
B = 8
N = 2048
NO = N // P            # 16 row tiles
DIN = 256
DOUT = 256
KO = DIN // P          # 2 k tiles for H @ W
W3 = 3 * DOUT
D2 = DOUT + 2          # moving block width with ones column


def build():
    nc = bacc.Bacc("TRN2", target_bir_lowering=False)
    G = nc.declare_dram_parameter("G", [N, N], F32, isOutput=False)
    H = nc.declare_dram_parameter("H", [N, DIN], F32, isOutput=False)
    W = nc.declare_dram_parameter("W", [DIN, DOUT], F32, isOutput=False)
    out = nc.declare_dram_parameter("out", [N, W3], F32, isOutput=True)

    G_r = G.rearrange("(o p) j -> p o j", p=P)
    H_r = H.rearrange("(o p) d -> p o d", p=P)
    W_r = W.rearrange("(o p) d -> p o d", p=P)
    out_r = out.rearrange("(o p) d -> p o d", p=P)

    with tile.TileContext(nc) as tc:
        with (
            tc.tile_pool(name="const", bufs=1) as const,
            tc.tile_pool(name="gn", bufs=1) as gn,
            tc.tile_pool(name="gt", bufs=1) as gt,
            tc.tile_pool(name="gstage", bufs=4) as gstage,
            tc.tile_pool(name="nmp", bufs=1) as nmp,
            tc.tile_pool(name="t2pp", bufs=1) as t2pp,
            tc.tile_pool(name="stage", bufs=3) as stage,
        ):
            # ---- constants ----
            ident_f32 = const.tile([P, P], F32)
            make_identity(nc, ident_f32)
            ident16 = const.tile([P, P], BF16)
            nc.vector.tensor_copy(ident16, ident_f32)
            rs_sb = const.tile([P, NO, 1], F32)
            rsinv = const.tile([P, NO, 1], F32)
            csinv = const.tile([P, NO, 1], F32)

            # ---- persistent bf16 tensors ----
            g16 = [gn.tile([P, N], BF16, tag=f"g{o}", name=f"g{o}") for o in range(NO)]
            strips = gt.tile([P, NO, N], BF16)     # [col-in-block, jt, row]
            # moving operand per k tile: [ones(0:2) | N_e(2:258) | T1'(258:514)]
            nm = nmp.tile([P, NO, 514], BF16)
            nc.vector.memset(nm[:, :, 0:2], 1.0)
            t2p = t2pp.tile([P, NO, DOUT], BF16)
            diag16 = t2pp.tile([P, NO, P], BF16)   # diag(rs) blocks
            cs_sb = const.tile([P, NO, 1], F32)

            # ---- input DMAs: W, H first (sync queue head), then G.
            # H stages inside the G ring (2 slots, recycled for G after). ----
            hs = []
            for i in range(2):
                h_sl = gstage.tile([P, N], F32, tag="gs", name=f"hs{i}")
                nc.sync.dma_start(
                    h_sl.rearrange("p (a b) -> p a b", a=8),
                    H_r[:, 8 * i:8 * (i + 1), :],
                )
                hs.append(h_sl)
            w_st = const.tile([P, KO, DOUT], F32)
            nc.sync.dma_start(w_st, W_r)

            gs_tiles = []
            for o in range(NO):
                gs = gstage.tile([P, N], F32, tag="gs", name=f"gs{o}")
                nc.sync.dma_start(gs[:, 0:N // 2], G_r[:, o, 0:N // 2])
                nc.sync.dma_start(gs[:, N // 2:N], G_r[:, o, N // 2:N])
                gs_tiles.append(gs)

            # ---- H @ W -> nm[:, :, 0:DOUT] ----
            w16 = const.tile([P, KO, DOUT], BF16)
            nc.vector.tensor_copy(w16, w_st)
            lpools = tc.tile_pool(name="psT", bufs=1, space="PSUM")
            psTp = lpools.__enter__()
            lpools2 = tc.tile_pool(name="psB1", bufs=2, space="PSUM")
            psB1 = lpools2.__enter__()
            with (
                tc.tile_pool(name="h16p", bufs=4) as h16p,
                tc.tile_pool(name="htp", bufs=3) as htp,
                tc.tile_pool(name="ps_h", bufs=1, space="PSUM") as ps_h,
                tc.tile_pool(name="ps_ne", bufs=2, space="PSUM") as ps_ne,
            ):
                psh = ps_h.tile([P, 8, P], BF16)
                hts = {}
                h16s = {}
                for t in range(NO + 1):
                    if t < NO:
                        if t % 2 == 0:   # cast two H tiles at once, alt engines
                            h16_t = h16p.tile([P, 2, DIN], BF16, tag="h16")
                            sl = hs[t // 8].rearrange("p (a b) -> p a b", a=8)
                            src_ap = sl[:, t % 8:t % 8 + 2, :]
                            if t % 4 == 0:
                                nc.scalar.copy(h16_t, src_ap)
                            else:
                                nc.vector.tensor_copy(h16_t, src_ap)
                            h16s[t] = h16_t
                        h16_c = h16s[t - t % 2][:, t % 2, :]
                        ht_t = htp.tile([P, KO, P], BF16, tag="ht")
                        s = (2 * t) % 8
                        for kt in range(KO):
                            nc.tensor.transpose(
                                psh[:, s + kt, :],
                                h16_c[:, kt * P:(kt + 1) * P],
                                ident16,
                            )
                        if t % 2 == 0:
                            nc.vector.tensor_copy(ht_t, psh[:, s:s + 2, :])
                        else:
                            nc.scalar.copy(ht_t, psh[:, s:s + 2, :])
                        hts[t] = ht_t
                    if t >= 1:
                        u = t - 1
                        ht_u = hts.pop(u)
                        pne = ps_ne.tile([P, DOUT], F32, tag="pne")
                        for kt in range(KO):
                            nc.tensor.matmul(
                                pne,
                                ht_u[:, kt, :],
                                w16[:, kt, :],
                                start=(kt == 0),
                                stop=(kt == KO - 1),
                            )
                        if u % 2 == 0:
                            nc.vector.tensor_copy(nm[:, u, 2:D2], pne)
                        else:
                            nc.scalar.copy(nm[:, u, 2:D2], pne)

            # ---- phase L: per arriving G tile: cast, transposes, B1 ----
            if True:
                psT = psTp.tile([P, 16, P], BF16)  # 16 transpose slots = 2 banks

                def b1_pass(u):
                    # [rs | T1] = G[u rows, :] @ [ones | N_e]
                    pb1 = psB1.tile([P, D2], F32, tag="pb1")
                    for jm in range(NO):
                        nc.tensor.matmul(
                            pb1,
                            strips[:, jm, u * P:(u + 1) * P],
                            nm[:, jm, 0:D2],
                            start=(jm == 0),
                            stop=(jm == NO - 1),
                        )
                    nc.vector.reciprocal(rsinv[:, u, :], pb1[:, 0:1])
                    # diag(rs) block for the fused pass's T1*rs accumulation
                    nc.vector.tensor_scalar_mul(
                        diag16[:, u, :], ident16, pb1[:, 0:1]
                    )
                    # nm[.., 258:514] = T1' = T1 * rsinv  (ACT, scale port)
                    nc.scalar.activation(
                        nm[:, u, D2:D2 + DOUT], pb1[:, 2:D2], COPY,
                        scale=rsinv[:, u, 0:1],
                    )

                CA = 12 * P   # ACT casts [0:1536], GpSimd the rest

                def cast_tile(o):
                    nc.scalar.copy(g16[o][:, 0:CA], gs_tiles[o][:, 0:CA])
                    nc.gpsimd.tensor_copy(g16[o][:, CA:N], gs_tiles[o][:, CA:N])

                cast_tile(0)
                cast_tile(1)
                for o in range(NO):
                    if o + 2 < NO:
                        cast_tile(o + 2)
                    for q in range(2):       # 8 transposes + 1 octo copy, x2
                        for jt in range(8 * q, 8 * q + 8):
                            nc.tensor.transpose(
                                psT[:, jt, :],
                                g16[o][:, jt * P:(jt + 1) * P],
                                ident16,
                            )
                        src = psT[:, 8 * q:8 * q + 8, :]
                        dst = strips[:, 8 * q:8 * q + 8, o * P:(o + 1) * P]
                        nc.vector.tensor_copy(dst, src)
                    if o >= 3:
                        b1_pass(o - 3)
                for u in range(NO - 3, NO):
                    b1_pass(u)

            # ---- fused pass: [T2+diag(rs)@T1' | out2raw] = G.T @ [N_e|T1'] ----
            # cs comes from DVE strip reductions, pipelined two tiles ahead.
            with (
                tc.tile_pool(name="psAC", bufs=3, space="PSUM") as psAC,
            ):
                def cs_reduce(j):
                    nc.vector.reduce_sum(cs_sb[:, j, :], strips[:, j, :], axis=AX)

                cs_reduce(0)
                cs_reduce(1)

                def ac_finish(j, pa):
                    # pa[:,0:DOUT] += T1*rs  (diag matmul; after t2p read T2)
                    nc.tensor.matmul(
                        pa[:, 0:DOUT],
                        diag16[:, j, :],
                        nm[:, j, D2:D2 + DOUT],
                        start=False,
                        stop=True,
                        skip_group_check=True,
                    )
                    o1 = stage.tile([P, DOUT], F32, tag="o1")
                    nc.scalar.activation(o1, pa[:, 0:DOUT], RELU, scale=0.5)
                    nc.gpsimd.dma_start(out_r[:, j, 0:DOUT], o1)
                    o2 = stage.tile([P, DOUT], F32, tag="o2")
                    nc.scalar.activation(o2, pa[:, DOUT:2 * DOUT], RELU)
                    nc.sync.dma_start(out_r[:, j, DOUT:2 * DOUT], o2)

                pas = {}
                for jt in range(NO):
                    pa = psAC.tile([P, 2 * DOUT], F32, tag="pa")
                    for kt in range(NO):
                        nc.tensor.matmul(
                            pa,
                            g16[kt][:, jt * P:(jt + 1) * P],
                            nm[:, kt, 2:2 + 2 * DOUT],
                            start=(kt == 0),
                            stop=(kt == NO - 1),
                        )
                    pas[jt] = pa
                    nc.vector.reciprocal(csinv[:, jt, :], cs_sb[:, jt, 0:1])
                    nc.vector.tensor_scalar_mul(
                        t2p[:, jt, :], pa[:, 0:DOUT], csinv[:, jt, 0:1]
                    )
                    if jt + 2 < NO:
                        cs_reduce(jt + 2)
                    if jt >= 1:
                        ac_finish(jt - 1, pas.pop(jt - 1))
                ac_finish(NO - 1, pas.pop(NO - 1))

            lpools2.__exit__(None, None, None)
            lpools.__exit__(None, None, None)

            # ---- pass B2: out3 = relu(G @ T2') ----
            with tc.tile_pool(name="psB2", bufs=3, space="PSUM") as psB2:
                for it in range(NO):
                    pb = psB2.tile([P, DOUT], F32, tag="pb")
                    for jt in range(NO):
                        nc.tensor.matmul(
                            pb,
                            strips[:, jt, it * P:(it + 1) * P],
                            t2p[:, jt, :],
                            start=(jt == 0),
                            stop=(jt == NO - 1),
                        )
                    o3 = stage.tile([P, DOUT], F32, tag="o3")
                    nc.scalar.activation(o3, pb, RELU)
                    nc.sync.dma_start(out_r[:, it, 2 * DOUT:W3], o3)

    nc.compile()
    return nc


_NC = None


def _get_nc():
    global _NC
    if _NC is None:
        _NC = build()
    return _NC


def run(inputs: dict, trace: bool = False):
    """Run on 8 cores; returns (stacked_out [B,N,W3], BassKernelResults)."""
    H, G, W = inputs["H"], inputs["G"], inputs["W"]
    H = np.ascontiguousarray(H, dtype=np.float32)
    G = np.ascontiguousarray(G, dtype=np.float32)
    W = np.ascontiguousarray(W, dtype=np.float32)
    in_maps = [
        {"G": np.ascontiguousarray(G[b]), "H": np.ascontiguousarray(H[b]), "W": W}
        for b in range(B)
    ]
    nc = _get_nc()
    res = run_bass_kernel_spmd(nc, in_maps, core_ids=list(range(B)), trace=trace)
    out = np.stack([res.results[b]["out"] for b in range(B)], axis=0)
    return out, res


def kernel(H, G, W):
    out, _ = run({"H": H, "G": G, "W": W})
    return out


# revision 24
# speedup vs baseline: 1.0112x; 1.0077x over previous
"""DirectedGraphConvolution Trainium2 kernel (bf16, software-pipelined).

Per batch element b (one per NeuronCore, 8 total, data-parallel):
    N_e = H @ W                          [n, dout]
    T1  = G  @ N_e ; T2 = G.T @ N_e
    rs  = G.sum(-1); cs = G.sum(-2)
    out = [ relu(0.5*(T1 + T2)),
            relu(G.T @ (T1 / rs[:,None])),
            relu(G  @ (T2 / cs[:,None])) ]

All matmuls run in bf16 (f32 PSUM, ~1 cyc/row; scale-relative error
~3e-3, well inside the 2e-2 gate).  G streams in f32 on the sync queue
through a 4-slot staging ring (H borrows the first two slots).  Per
tile, the f32->bf16 cast is split ACT/GpSimd and runs two tiles ahead;
the PE transposes all 16 blocks (bf16 transposes are 1 cyc/row) into a
persistent G.T strip tensor via two 8-block strided PSUM->SBUF copies
on DVE (16-bit DVE copies run 2 elem/cyc), and B1 = G @ [ones|N_e]
follows three tiles behind, keeping the PE busy through the whole DMA
window.  B1's ones column yields rs; rs also becomes a diag(rs) bf16
block (DVE scales the identity) and T1' = T1/rs lands next to N_e in
the shared moving tensor nm = [ones | N_e | T1'].

Post-load, passes A and C share their stationaries (natural G blocks),
so they run fused with the 512-wide moving [N_e | T1'] -- wide enough
to hide the 115 ns LDWEIGHTS per stationary that rate-limits 256-wide
matmuls.  Epilogue per column tile: cs from DVE strip reductions
(pipelined two tiles ahead), T2' = T2/cs, then one diag(rs) @ T1'
matmul accumulates T1 into the PSUM tile so out1 = relu(0.5*psum)
needs no DVE add.  B2 = G @ T2' reuses the strips (G is transposed
exactly once).  Outputs stream per-tile on the sync/gpsimd queues.
"""

import numpy as np
import concourse.bass as bass
import concourse.mybir as mybir
import concourse.tile as tile
from concourse import bacc
from concourse.bass_utils import run_bass_kernel_spmd
from concourse.masks import make_identity

F32 = mybir.dt.float32
BF16 = mybir.dt.bfloat16
COPY = mybir.ActivationFunctionType.Copy
RELU = mybir.ActivationFunctionType.Relu
AX = mybir.AxisListType.X

P = 128
B = 8
N = 2048
NO = N // P            # 16 row tiles
DIN = 256
DOUT = 256
KO = DIN // P          # 2 k tiles for H @ W
W3 = 3 * DOUT
D2 = DOUT + 2          # moving block width with ones column


def build():
    nc = bacc.Bacc("TRN2", target_bir_lowering=False)
    G = nc.declare_dram_parameter("G", [N, N], F32, isOutput=False)
    H = nc.declare_dram_parameter("H", [N, DIN], F32, isOutput=False)
    W = nc.declare_dram_parameter("W", [DIN, DOUT], F32, isOutput=False)
    out = nc.declare_dram_parameter("out", [N, W3], F32, isOutput=True)

    G_r = G.rearrange("(o p) j -> p o j", p=P)
    H_r = H.rearrange("(o p) d -> p o d", p=P)
    W_r = W.rearrange("(o p) d -> p o d", p=P)
    out_r = out.rearrange("(o p) d -> p o d", p=P)

    with tile.TileContext(nc) as tc:
        with (
            tc.tile_pool(name="const", bufs=1) as const,
            tc.tile_pool(name="gn", bufs=1) as gn,
            tc.tile_pool(name="gt", bufs=1) as gt,
            tc.tile_pool(name="gstage", bufs=4) as gstage,
            tc.tile_pool(name="nmp", bufs=1) as nmp,
            tc.tile_pool(name="t2pp", bufs=1) as t2pp,
            tc.tile_pool(name="stage", bufs=3) as stage,
        ):
            # ---- constants ----
            ident_f32 = const.tile([P, P], F32)
            make_identity(nc, ident_f32)
            ident16 = const.tile([P, P], BF16)
            nc.vector.tensor_copy(ident16, ident_f32)
            rs_sb = const.tile([P, NO, 1], F32)
            rsinv = const.tile([P, NO, 1], F32)
            csinv = const.tile([P, NO, 1], F32)

            # ---- persistent bf16 tensors ----
            g16 = [gn.tile([P, N], BF16, tag=f"g{o}", name=f"g{o}") for o in range(NO)]
            strips = gt.tile([P, NO, N], BF16)     # [col-in-block, jt, row]
            # moving operand per k tile: [ones(0:2) | N_e(2:258) | T1'(258:514)]
            nm = nmp.tile([P, NO, 514], BF16)
            nc.vector.memset(nm[:, :, 0:2], 1.0)
            t2p = t2pp.tile([P, NO, DOUT], BF16)
            diag16 = t2pp.tile([P, NO, P], BF16)   # diag(rs) blocks
            cs_sb = const.tile([P, NO, 1], F32)

            # ---- input DMAs: W, H first (sync queue head), then G.
            # H stages inside the G ring (2 slots, recycled for G after). ----
            hs = []
            for i in range(2):
                h_sl = gstage.tile([P, N], F32, tag="gs", name=f"hs{i}")
                nc.sync.dma_start(
                    h_sl.rearrange("p (a b) -> p a b", a=8),
                    H_r[:, 8 * i:8 * (i + 1), :],
                )
                hs.append(h_sl)
            w_st = const.tile([P, KO, DOUT], F32)
            nc.sync.dma_start(w_st, W_r)

            gs_tiles = []
            for o in range(NO):
                gs = gstage.tile([P, N], F32, tag="gs", name=f"gs{o}")
                nc.sync.dma_start(gs[:, 0:N // 2], G_r[:, o, 0:N // 2])
                nc.sync.dma_start(gs[:, N // 2:N], G_r[:, o, N // 2:N])
                gs_tiles.append(gs)

            # ---- H @ W -> nm[:, :, 0:DOUT] ----
            w16 = const.tile([P, KO, DOUT], BF16)
            nc.vector.tensor_copy(w16, w_st)
            lpools = tc.tile_pool(name="psT", bufs=1, space="PSUM")
            psTp = lpools.__enter__()
            lpools2 = tc.tile_pool(name="psB1", bufs=2, space="PSUM")
            psB1 = lpools2.__enter__()
            with (
                tc.tile_pool(name="h16p", bufs=4) as h16p,
                tc.tile_pool(name="htp", bufs=3) as htp,
                tc.tile_pool(name="ps_h", bufs=1, space="PSUM") as ps_h,
                tc.tile_pool(name="ps_ne", bufs=2, space="PSUM") as ps_ne,
            ):
                psh = ps_h.tile([P, 8, P], BF16)
                hts = {}
                h16s = {}
                for t in range(NO + 1):
                    if t < NO:
                        if t % 2 == 0:   # cast two H tiles at once, alt engines
                            h16_t = h16p.tile([P, 2, DIN], BF16, tag="h16")
                            sl = hs[t // 8].rearrange("p (a b) -> p a b", a=8)
                            src_ap = sl[:, t % 8:t % 8 + 2, :]
                            if t % 4 == 0:
                                nc.scalar.copy(h16_t, src_ap)
                            else:
                                nc.vector.tensor_copy(h16_t, src_ap)
                            h16s[t] = h16_t
                        h16_c = h16s[t - t % 2][:, t % 2, :]
                        ht_t = htp.tile([P, KO, P], BF16, tag="ht")
                        s = (2 * t) % 8
                        for kt in range(KO):
                            nc.tensor.transpose(
                                psh[:, s + kt, :],
                                h16_c[:, kt * P:(kt + 1) * P],
                                ident16,
                            )
                        if t % 2 == 0:
                            nc.vector.tensor_copy(ht_t, psh[:, s:s + 2, :])
                        else:
                            nc.scalar.copy(ht_t, psh[:, s:s + 2, :])
                        hts[t] = ht_t
                    if t >= 1:
                        u = t - 1
                        ht_u = hts.pop(u)
                        pne = ps_ne.tile([P, DOUT], F32, tag="pne")
                        for kt in range(KO):
                            nc.tensor.matmul(
                                pne,
                                ht_u[:, kt, :],
                                w16[:, kt, :],
                                start=(kt == 0),
                                stop=(kt == KO - 1),
                            )
                        if u % 2 == 0:
                            nc.vector.tensor_copy(nm[:, u, 2:D2], pne)
                        else:
                            nc.scalar.copy(nm[:, u, 2:D2], pne)

            # ---- phase L: per arriving G tile: cast, transposes, B1 ----
            if True:
                psT = psTp.tile([P, 24, P], BF16)  # 24 slots, 3 rotating groups

                def b1_pass(u):
                    # [rs | T1] = G[u rows, :] @ [ones | N_e]
                    pb1 = psB1.tile([P, D2], F32, tag="pb1")
                    for jm in range(NO):
                        nc.tensor.matmul(
                            pb1,
                            strips[:, jm, u * P:(u + 1) * P],
                            nm[:, jm, 0:D2],
                            start=(jm == 0),
                            stop=(jm == NO - 1),
                        )
                    nc.vector.reciprocal(rsinv[:, u, :], pb1[:, 0:1])
                    # diag(rs) block for the fused pass's T1*rs accumulation
                    nc.vector.tensor_scalar_mul(
                        diag16[:, u, :], ident16, pb1[:, 0:1]
                    )
                    # nm[.., 258:514] = T1' = T1 * rsinv  (ACT, scale port)
                    nc.scalar.activation(
                        nm[:, u, D2:D2 + DOUT], pb1[:, 2:D2], COPY,
                        scale=rsinv[:, u, 0:1],
                    )

                CA = 12 * P   # ACT casts [0:1536], GpSimd the rest

                def cast_tile(o):
                    nc.scalar.copy(g16[o][:, 0:CA], gs_tiles[o][:, 0:CA])
                    nc.gpsimd.tensor_copy(g16[o][:, CA:N], gs_tiles[o][:, CA:N])

                cast_tile(0)
                cast_tile(1)
                for o in range(NO):
                    if o + 2 < NO:
                        cast_tile(o + 2)
                    for q in range(2):       # 8 transposes + 1 octo copy, x2
                        base = 8 * ((2 * o + q) % 3)
                        for j in range(8):
                            jt = 8 * q + j
                            nc.tensor.transpose(
                                psT[:, base + j, :],
                                g16[o][:, jt * P:(jt + 1) * P],
                                ident16,
                            )
                        src = psT[:, base:base + 8, :]
                        dst = strips[:, 8 * q:8 * q + 8, o * P:(o + 1) * P]
                        nc.vector.tensor_copy(dst, src)
                    if o >= 3:
                        b1_pass(o - 3)
                for u in range(NO - 3, NO):
                    b1_pass(u)

            # ---- fused pass: [T2+diag(rs)@T1' | out2raw] = G.T @ [N_e|T1'] ----
            # cs comes from DVE strip reductions, pipelined two tiles ahead.
            with (
                tc.tile_pool(name="psAC", bufs=3, space="PSUM") as psAC,
            ):
                def cs_reduce(j):
                    nc.vector.reduce_sum(cs_sb[:, j, :], strips[:, j, :], axis=AX)

                cs_reduce(0)
                cs_reduce(1)

                def ac_finish(j, pa):
                    # pa[:,0:DOUT] += T1*rs  (diag matmul; after t2p read T2)
                    nc.tensor.matmul(
                        pa[:, 0:DOUT],
                        diag16[:, j, :],
                        nm[:, j, D2:D2 + DOUT],
                        start=False,
                        stop=True,
                        skip_group_check=True,
                    )
                    o1 = stage.tile([P, DOUT], F32, tag="o1")
                    nc.scalar.activation(o1, pa[:, 0:DOUT], RELU, scale=0.5)
                    nc.gpsimd.dma_start(out_r[:, j, 0:DOUT], o1)
                    o2 = stage.tile([P, DOUT], F32, tag="o2")
                    nc.scalar.activation(o2, pa[:, DOUT:2 * DOUT], RELU)
                    nc.sync.dma_start(out_r[:, j, DOUT:2 * DOUT], o2)

                pas = {}
                for jt in range(NO):
                    pa = psAC.tile([P, 2 * DOUT], F32, tag="pa")
                    for kt in range(NO):
                        nc.tensor.matmul(
                            pa,
                            g16[kt][:, jt * P:(jt + 1) * P],
                            nm[:, kt, 2:2 + 2 * DOUT],
                            start=(kt == 0),
                            stop=(kt == NO - 1),
                        )
                    pas[jt] = pa
                    nc.vector.reciprocal(csinv[:, jt, :], cs_sb[:, jt, 0:1])
                    nc.vector.tensor_scalar_mul(
                        t2p[:, jt, :], pa[:, 0:DOUT], csinv[:, jt, 0:1]
                    )
                    if jt + 2 < NO:
                        cs_reduce(jt + 2)
                    if jt >= 1:
                        ac_finish(jt - 1, pas.pop(jt - 1))
                ac_finish(NO - 1, pas.pop(NO - 1))

            lpools2.__exit__(None, None, None)
            lpools.__exit__(None, None, None)

            # ---- pass B2: out3 = relu(G @ T2') ----
            with tc.tile_pool(name="psB2", bufs=3, space="PSUM") as psB2:
                for it in range(NO):
                    pb = psB2.tile([P, DOUT], F32, tag="pb")
                    for jt in range(NO):
                        nc.tensor.matmul(
                            pb,
                            strips[:, jt, it * P:(it + 1) * P],
                            t2p[:, jt, :],
                            start=(jt == 0),
                            stop=(jt == NO - 1),
                        )
                    o3 = stage.tile([P, DOUT], F32, tag="o3")
                    nc.scalar.activation(o3, pb, RELU)
                    nc.sync.dma_start(out_r[:, it, 2 * DOUT:W3], o3)

    nc.compile()
    return nc


_NC = None


def _get_nc():
    global _NC
    if _NC is None:
        _NC = build()
    return _NC


def run(inputs: dict, trace: bool = False):
    """Run on 8 cores; returns (stacked_out [B,N,W3], BassKernelResults)."""
    H, G, W = inputs["H"], inputs["G"], inputs["W"]
    H = np.ascontiguousarray(H, dtype=np.float32)
    G = np.ascontiguousarray(G, dtype=np.float32)
    W = np.ascontiguousarray(W, dtype=np.float32)
    in_maps = [
        {"G": np.ascontiguousarray(G[b]), "H": np.ascontiguousarray(H[b]), "W": W}
        for b in range(B)
    ]
    nc = _get_nc()
    res = run_bass_kernel_spmd(nc, in_maps, core_ids=list(range(B)), trace=trace)
    out = np.stack([res.results[b]["out"] for b in range(B)], axis=0)
    return out, res


def kernel(H, G, W):
    out, _ = run({"H": H, "G": G, "W": W})
    return out
